# revision 1
# baseline (speedup 1.0000x reference)
"""Multi-head attention Trainium2 kernel (8 NeuronCores).

Sharding: core c owns batch b = c//2 and heads h0 = (c%2)*6 .. h0+6 (tensor
parallel over heads x data parallel over batch). Each core computes its 6
heads' attention and a partial output projection; the host sums the two
partial projections per batch element and adds the output bias.

Per-core layout (all matmuls in float32r, fp32 PSUM accumulation):
  xt  [D=768, S=2048]   x[b] transposed on host, D on partitions
  Qt/Kt [384, 2048]     (h e) on partitions, computed as Wq^T @ x^T
  V   [S, 576]          keys on partitions; per head pair: [V_a|ones|V_b]
  scores^T [keys, q]    per 128-key chunk, via lhsT=Kt slice (K=64)
  exp on ScalarE PSUM->SBUF, fused 1/8 scale
  ctx^T += [V_h|ones]^T @ exp: 64 psum rows of unnormalized ctx +
                        64 rows of replicated softmax denominator
  normalize on VectorE, project with Wo chunks, DMA partial out.
"""
import sys

sys.path.insert(0, "/opt/trn_rl_repo")

from contextlib import ExitStack

import numpy as np

import concourse.bacc as bacc
import concourse.bass as bass
import concourse.mybir as mybir
import concourse.tile as tile
from concourse.bass_utils import run_bass_kernel_spmd

f32 = mybir.dt.float32
f32r = mybir.dt.float32r
AF = mybir.ActivationFunctionType
ALU = mybir.AluOpType

B, S, D = 4, 2048, 768
H, E = 12, 64
HL = 6              # heads per core
F = HL * E          # 384: local concat-head feature dim
ND = D // 128       # 6 contraction chunks over D
NF = F // 128       # 3 chunks over F
NK = S // 128       # 16 key chunks
QB = 512            # q block (matmul moving free dim)
NQB = S // QB       # 4
KG = 2              # key chunks per exp group
OW = 32            # ones block width (sums replicated OW rows)
VW = NF * (2 * E + OW)  # V tile width: 3 pairs x [V_a|ones|V_b]
NCORES = 8

_NC = None


def _build():
    nc = bacc.Bacc()
    xt_d = nc.declare_dram_parameter("xt", [D, S], f32r, isOutput=False)
    wq_d = nc.declare_dram_parameter("wq", [D, F], f32r, isOutput=False)
    wk_d = nc.declare_dram_parameter("wk", [D, F], f32r, isOutput=False)
    wv_d = nc.declare_dram_parameter("wv", [D, F], f32r, isOutput=False)
    wo_d = nc.declare_dram_parameter("wo", [F, D], f32r, isOutput=False)
    bq_d = nc.declare_dram_parameter("bq", [F, 1], f32, isOutput=False)
    bk_d = nc.declare_dram_parameter("bk", [F, 1], f32, isOutput=False)
    bv_d = nc.declare_dram_parameter("bv", [1, F], f32, isOutput=False)
    ones_d = nc.declare_dram_parameter("ones", [1, NF * OW], f32r, isOutput=False)
    y_d = nc.declare_dram_parameter("y", [S, D], f32, isOutput=True)

    with tile.TileContext(nc) as tc, ExitStack() as ctx:
        big = ctx.enter_context(tc.tile_pool(name="big", bufs=16))
        vpool = ctx.enter_context(tc.tile_pool(name="vpool", bufs=NK))
        wpool = ctx.enter_context(tc.tile_pool(name="wpool", bufs=9))
        wopool = ctx.enter_context(tc.tile_pool(name="wopool", bufs=3))
        epool = ctx.enter_context(tc.tile_pool(name="epool", bufs=2))
        opool = ctx.enter_context(tc.tile_pool(name="opool", bufs=3))
        npool = ctx.enter_context(tc.tile_pool(name="npool", bufs=1))
        cpool = ctx.enter_context(tc.tile_pool(name="cpool", bufs=1))
        psA = ctx.enter_context(tc.tile_pool(name="psA", bufs=4, space="PSUM"))
        pssc = ctx.enter_context(tc.tile_pool(name="pssc", bufs=2, space="PSUM"))

        # --- constant/bias tiles ---
        bq_sb = cpool.tile([128, NF], f32, name="bq_sb", tag="bq")
        nc.sync.dma_start(out=bq_sb, in_=bq_d.rearrange("(m p) o -> p m o", p=128))
        bk_sb = cpool.tile([128, NF], f32, name="bk_sb", tag="bk")
        nc.sync.dma_start(out=bk_sb, in_=bk_d.rearrange("(m p) o -> p m o", p=128))
        # bv broadcast across partitions via 0-stride DRAM read
        bv_bc = cpool.tile([128, F], f32, name="bv_bc", tag="bv")
        bv_src = bv_d[0:1, :]
        bv_ap = bass.AP(tensor=bv_src.tensor, offset=bv_src.offset,
                        ap=[[0, 128]] + list(bv_src.ap)[1:])
        nc.sync.dma_start(out=bv_bc, in_=bv_ap)

        # --- input/weight tiles: gpsimd DMA casts f32 -> f32r on the fly ---
        xt_t = []
        for kd in range(ND):
            t = big.tile([128, S], f32r, tag="big", name=f"xt{kd}")
            eng = nc.sync if kd % 2 == 0 else nc.scalar
            eng.dma_start(out=t, in_=xt_d[kd * 128:(kd + 1) * 128, :])
            xt_t.append(t)
        v_t = []
        for mk in range(NK):
            t = vpool.tile([128, VW], f32r, tag="v", name=f"v{mk}")
            t3o = t[:].rearrange("p (pair c) -> p pair c", c=2 * E + OW)
            src = xt_t[0][:, 0:NF * OW].rearrange("p (pair e) -> p pair e", e=OW)
            nc.vector.tensor_scalar(
                t3o[:, :, E:E + OW], src, 0.0, 1.0,
                op0=ALU.mult, op1=ALU.add)
            v_t.append(t)
        wq_t, wk_t, wv_t = [], [], []
        for nm, dd, lst in (("wv", wv_d, wv_t), ("wk", wk_d, wk_t), ("wq", wq_d, wq_t)):
            for kd in range(ND):
                t = wpool.tile([128, F], f32r, tag="w", name=f"{nm}{kd}")
                nc.sync.dma_start(out=t, in_=dd[kd * 128:(kd + 1) * 128, :])
                lst.append(t)
        wo_t = []
        for kf in range(NF):
            t = wopool.tile([128, D], f32r, tag="wo", name=f"wo{kf}")
            nc.sync.dma_start(out=t, in_=wo_d[kf * 128:(kf + 1) * 128, :])
            wo_t.append(t)

        # --- Qt (zero-padded per head, K=128 scores), Kt paired ---
        qt_t = [big.tile([128, S], f32r, tag="big", name=f"qt{h}") for h in range(2 * NF)]
        kt_t = [big.tile([128, S], f32r, tag="big", name=f"kt{m}") for m in range(NF)]
        for m in range(NF):
            nc.vector.tensor_scalar_mul(qt_t[2 * m][E:128, :], xt_t[0][E:128, :], 0.0)
            nc.vector.tensor_scalar_mul(qt_t[2 * m + 1][0:E, :], xt_t[0][0:E, :], 0.0)

        def emit_k_group(m, nq):
            ps = psA.tile([128, QB], f32, tag="bank", name=f"p1k_{m}_{nq}")
            for kd in range(ND):
                nc.tensor.matmul(
                    ps[:, :],
                    lhsT=wk_t[kd][:, m * 128:(m + 1) * 128],
                    rhs=xt_t[kd][:, nq * QB:(nq + 1) * QB],
                    start=(kd == 0), stop=(kd == ND - 1),
                )
            nc.vector.tensor_scalar_add(
                kt_t[m][:, nq * QB:(nq + 1) * QB], ps[:, :], bk_sb[:, m:m + 1])

        def emit_q_group(m, nq):
            sl = slice(nq * QB, (nq + 1) * QB)
            ps = psA.tile([128, QB], f32, tag="bank", name=f"p1q_{m}_{nq}")
            for kd in range(ND):
                nc.tensor.matmul(
                    ps[:, :],
                    lhsT=wq_t[kd][:, m * 128:(m + 1) * 128],
                    rhs=xt_t[kd][:, nq * QB:(nq + 1) * QB],
                    start=(kd == 0), stop=(kd == ND - 1),
                )
            nc.vector.tensor_scalar_add(
                qt_t[2 * m][0:E, sl], ps[0:E, :], bq_sb[0:E, m:m + 1])
            nc.vector.tensor_scalar_add(
                qt_t[2 * m + 1][E:128, sl], ps[E:128, :], bq_sb[E:128, m:m + 1])

        # --- V first: attention ctx needs all of it ---
        for mk in range(NK):
            ps = psA.tile([128, F], f32, tag="bank", name=f"p1v_{mk}", padded_shape=[128, QB])
            for kd in range(ND):
                nc.tensor.matmul(
                    ps[:, :],
                    lhsT=xt_t[kd][:, mk * 128:(mk + 1) * 128],
                    rhs=wv_t[kd][:, :],
                    start=(kd == 0), stop=(kd == ND - 1),
                )
            t = v_t[mk]
            ps3 = ps.rearrange("p (pair hh e) -> p pair hh e", hh=2, e=E)
            bv3 = bv_bc.rearrange("p (pair hh e) -> p pair hh e", hh=2, e=E)
            t3 = t[:].rearrange("p (pair c) -> p pair c", c=2 * E + OW)
            nc.vector.tensor_tensor(
                t3[:, :, 0:E], ps3[:, :, 0, :], bv3[:, :, 0, :], op=ALU.add)
            nc.vector.tensor_tensor(
                t3[:, :, E + OW:2 * E + OW], ps3[:, :, 1, :], bv3[:, :, 1, :], op=ALU.add)

        # pair-0 K/Q projections up front; pairs 1-2 are emitted as fillers
        # inside head-pair-0's attention units (the PE has slack there while
        # ScalarE grinds through exp).
        for m in range(NF):
            for nq in range(NQB):
                emit_k_group(m, nq)
        for m in range(NF):
            for nq in range(NQB):
                emit_q_group(m, nq)
        qk_fill = []

        # --- output projection, per head pair, DMA-accumulated into y ---
        ctxt_t = [big.tile([128, S], f32r, tag="big", name=f"ctxt{m}") for m in range(NF)]


        # --- attention ---
        for hp in range(NF):          # head pair (partitions 0:64 / 64:128)
            for nq in range(NQB):
                cps = [
                    psA.tile([128, QB], f32, tag="bank", name=f"c{hp}_{nq}_{hh}")
                    for hh in range(2)
                ]
                for _ in range(2):
                    if qk_fill:
                        m, fnq, w = qk_fill.pop(0)
                        (emit_k_group if w == "k" else emit_q_group)(m, fnq)
                def normalize(hh):
                    # head a: ctx rows 0:64, sums 64:64+OW; head b: sums
                    # rows 0:OW, ctx OW:OW+64.
                    c0 = 0 if hh == 0 else OW
                    s0 = E if hh == 0 else 0
                    r = npool.tile([OW, QB], f32, tag="n",
                                   name=f"n{hp}_{nq}_{hh}", bufs=2)
                    nc.vector.reciprocal(r[:], cps[hh][s0:s0 + OW, :])
                    for half in range(E // OW):
                        nc.vector.tensor_tensor(
                            ctxt_t[hp][hh * E + half * OW:hh * E + (half + 1) * OW,
                                       nq * QB:(nq + 1) * QB],
                            cps[hh][c0 + half * OW:c0 + (half + 1) * OW, :],
                            r[:], op=ALU.mult,
                        )

                for g in range(NK // KG):
                    for hh in range(2):
                        sps = pssc.tile([128, KG * QB], f32, tag="sc",
                                        name=f"s{hp}_{nq}_{g}_{hh}")
                        for j in range(KG):
                            mk = g * KG + j
                            nc.tensor.matmul(
                                sps[:, j * QB:(j + 1) * QB],
                                lhsT=kt_t[hp][:, mk * 128:(mk + 1) * 128],
                                rhs=qt_t[2 * hp + hh][:, nq * QB:(nq + 1) * QB],
                                start=True, stop=True,
                            )
                        esb = epool.tile([128, KG * QB], f32r, tag="e",
                                         name=f"e{hp}_{nq}_{g}_{hh}")
                        nc.scalar.activation(esb[:], sps[:], AF.Exp, scale=0.125)
                        if g == 3 and hh == 0 and qk_fill and len(qk_fill) % 2 == 0:
                            m, fnq, w = qk_fill.pop(0)
                            (emit_k_group if w == "k" else emit_q_group)(m, fnq)
                        for j in range(KG):
                            mk = g * KG + j
                            base = hp * (2 * E + OW) + hh * E
                            nc.tensor.matmul(
                                cps[hh][0:E + OW, :],
                                lhsT=v_t[mk][:, base:base + E + OW],
                                rhs=esb[:, j * QB:(j + 1) * QB],
                                start=(g == 0 and j == 0),
                                stop=(g == NK // KG - 1 and j == KG - 1),
                            )
                        if g == NK // KG - 1:
                            normalize(hh)
        # tail projection: accumulate over all head pairs in PSUM
        for mq in range(NK):
            osb = opool.tile([128, D], f32, tag="o", name=f"ot{mq}")
            for piece, (c0, c1) in enumerate(((0, 512), (512, D))):
                ps = psA.tile([128, c1 - c0], f32, tag="bank",
                              name=f"o{piece}_{mq}", padded_shape=[128, QB])
                for kf in range(NF):
                    nc.tensor.matmul(
                        ps[:, :], lhsT=ctxt_t[kf][:, mq * 128:(mq + 1) * 128],
                        rhs=wo_t[kf][:, c0:c1],
                        start=(kf == 0), stop=(kf == NF - 1))
                nc.vector.tensor_copy(osb[:, c0:c1], ps[:, :])
            nc.sync.dma_start(out=y_d[mq * 128:(mq + 1) * 128, :], in_=osb[:])
    nc.compile()
    return nc


def _get_nc():
    global _NC
    if _NC is None:
        _NC = _build()
    return _NC


def kernel(x, Wq, bq, Wk, bk, Wv, bv, Wo, bo, _trace=False):
    x = np.asarray(x, dtype=np.float32)
    Wq = np.asarray(Wq, dtype=np.float32)
    bq = np.asarray(bq, dtype=np.float32)
    Wk = np.asarray(Wk, dtype=np.float32)
    bk = np.asarray(bk, dtype=np.float32)
    Wv = np.asarray(Wv, dtype=np.float32)
    bv = np.asarray(bv, dtype=np.float32)
    Wo = np.asarray(Wo, dtype=np.float32)
    bo = np.asarray(bo, dtype=np.float32)

    nc = _get_nc()
    in_maps = []
    for c in range(NCORES):
        b = c // 2
        h0 = (c % 2) * HL
        in_maps.append({
            "xt": np.ascontiguousarray(x[b].T),
            "wq": np.ascontiguousarray(Wq[h0:h0 + HL].transpose(1, 0, 2).reshape(D, F)),
            "wk": np.ascontiguousarray(Wk[h0:h0 + HL].transpose(1, 0, 2).reshape(D, F)),
            "wv": np.ascontiguousarray(Wv[h0:h0 + HL].transpose(1, 0, 2).reshape(D, F)),
            "wo": np.ascontiguousarray(Wo[h0 * E:(h0 + HL) * E]),
            "bq": np.ascontiguousarray(bq[h0:h0 + HL].reshape(F, 1)),
            "bk": np.ascontiguousarray(bk[h0:h0 + HL].reshape(F, 1)),
            "bv": np.ascontiguousarray(bv[h0:h0 + HL].reshape(1, F)),
            "ones": np.ones((1, NF * OW), np.float32),
        })
    res = run_bass_kernel_spmd(nc, in_maps, list(range(NCORES)), trace=_trace)
    out = np.empty((B, S, D), np.float32)
    for b in range(B):
        out[b] = res.results[2 * b]["y"] + res.results[2 * b + 1]["y"] + bo[None, :]
    if _trace:
        kernel.last_exec_time_ns = res.exec_time_ns
        kernel.last_results = res
    return out



# revision 15
# speedup vs baseline: 1.2993x; 1.2993x over previous
"""Multi-head attention Trainium2 kernel (8 NeuronCores).

Sharding: core c owns batch b = c//2 and heads h0 = (c%2)*6 .. h0+6 (tensor
parallel over heads x data parallel over batch). Each core computes its 6
heads' attention and a partial output projection; the host sums the two
partial projections per batch element and adds the output bias.

v2 design (vs f32r baseline):
  - all SBUF operands bf16 (DMA halved, DVE 2x eligible; PE cost unchanged)
  - V tiles per head pair laid out [V_a(64) | ones(64) | V_b(64)] so the
    ctx matmul produces 64 ctx rows + 64 replicated softmax denominators;
    normalize = reciprocal_approx_fast + one tensor_tensor per head
  - software-pipelined emission: scores(g+1) precedes ctx(g) in the PE
    queue so the PE never blocks behind ScalarE's exp
  - every QKV/O projection group is a "filler" pumped into the attention
    loop to absorb PE slack; dedicated PSUM pools (4+2+2 banks)
"""
import sys

sys.path.insert(0, "/opt/trn_rl_repo")

from contextlib import ExitStack

import ml_dtypes
import numpy as np

import concourse.bacc as bacc
import concourse.bass as bass
import concourse.mybir as mybir
import concourse.tile as tile
from concourse.bass_utils import run_bass_kernel_spmd

f32 = mybir.dt.float32
bf16 = mybir.dt.bfloat16
AF = mybir.ActivationFunctionType
ALU = mybir.AluOpType
BF = ml_dtypes.bfloat16

B, S, D = 4, 2048, 768
H, E = 12, 64
HL = 6              # heads per core
F = HL * E          # 384: local concat-head feature dim
ND = D // 128       # 6 contraction chunks over D
NF = F // 128       # 3 chunks over F (head pairs)
NK = S // 128       # 16 key chunks
QB = 512            # q block (matmul moving free dim)
NQB = S // QB       # 4
PW = 256            # per-pair V tile width [ones64 | V_a | ones64 | V_b]
VW = NF * PW        # 768
NCORES = 8

_NC = None


def _build(debug=False):
    nc = bacc.Bacc()
    xt_d = nc.declare_dram_parameter("xt", [D, S], bf16, isOutput=False)
    wq_d = nc.declare_dram_parameter("wq", [D, F], bf16, isOutput=False)
    wk_d = nc.declare_dram_parameter("wk", [D, F], bf16, isOutput=False)
    wv_d = nc.declare_dram_parameter("wv", [D, F], bf16, isOutput=False)
    wo_d = nc.declare_dram_parameter("wo", [F, D], bf16, isOutput=False)
    bq_d = nc.declare_dram_parameter("bq", [F, 1], f32, isOutput=False)
    bk_d = nc.declare_dram_parameter("bk", [F, 1], f32, isOutput=False)
    bv_d = nc.declare_dram_parameter("bv", [1, F], f32, isOutput=False)
    y_d = nc.declare_dram_parameter("y", [S, D], bf16, isOutput=True)

    with tile.TileContext(nc) as tc, ExitStack() as ctx:
        xpool = ctx.enter_context(tc.tile_pool(name="xpool", bufs=ND))
        qpool = ctx.enter_context(tc.tile_pool(name="qpool", bufs=2 * NF))
        kpool = ctx.enter_context(tc.tile_pool(name="kpool", bufs=NF))
        cxpool = ctx.enter_context(tc.tile_pool(name="cxpool", bufs=NF))
        vpool = ctx.enter_context(tc.tile_pool(name="vpool", bufs=NK))
        wpool = ctx.enter_context(tc.tile_pool(name="wpool", bufs=3 * ND))
        wopool = ctx.enter_context(tc.tile_pool(name="wopool", bufs=NF))
        epool = ctx.enter_context(tc.tile_pool(name="epool", bufs=4))
        rpool = ctx.enter_context(tc.tile_pool(name="rpool", bufs=2))
        opool = ctx.enter_context(tc.tile_pool(name="opool", bufs=2))
        cpool = ctx.enter_context(tc.tile_pool(name="cpool", bufs=3))
        pssc = ctx.enter_context(tc.tile_pool(name="pssc", bufs=2, space="PSUM"))
        psctx = ctx.enter_context(tc.tile_pool(name="psctx", bufs=2, space="PSUM"))
        pproj = ctx.enter_context(tc.tile_pool(name="pproj", bufs=2, space="PSUM"))

        # --- bias tiles ---
        bq_sb = cpool.tile([128, NF], f32, name="bq_sb", tag="bq")
        nc.sync.dma_start(out=bq_sb, in_=bq_d.rearrange("(m p) o -> p m o", p=128))
        bk_sb = cpool.tile([128, NF], f32, name="bk_sb", tag="bk")
        nc.sync.dma_start(out=bk_sb, in_=bk_d.rearrange("(m p) o -> p m o", p=128))
        # bv broadcast across partitions via 0-stride DRAM read
        bv_bc = cpool.tile([128, F], f32, name="bv_bc", tag="bv")
        bv_src = bv_d[0:1, :]
        bv_ap = bass.AP(tensor=bv_src.tensor, offset=bv_src.offset,
                        ap=[[0, 128]] + list(bv_src.ap)[1:])
        nc.scalar.dma_start(out=bv_bc, in_=bv_ap)

        # --- input/weight DMAs, spread over queues ---
        xt_t = []
        for kd in range(ND):
            t = xpool.tile([128, S], bf16, tag="x", name=f"xt{kd}")
            eng = (nc.sync, nc.scalar, nc.gpsimd)[kd % 3]
            eng.dma_start(out=t, in_=xt_d[kd * 128:(kd + 1) * 128, :])
            xt_t.append(t)
        wq_t, wk_t, wv_t = [], [], []
        for nm, dd, lst, eng in (("wk", wk_d, wk_t, nc.sync),
                                 ("wq", wq_d, wq_t, nc.scalar),
                                 ("wv", wv_d, wv_t, nc.gpsimd)):
            for kd in range(ND):
                t = wpool.tile([128, F], bf16, tag="w", name=f"{nm}{kd}")
                eng.dma_start(out=t, in_=dd[kd * 128:(kd + 1) * 128, :])
                lst.append(t)
        wo_t = []
        for kf in range(NF):
            t = wopool.tile([128, D], bf16, tag="wo", name=f"wo{kf}")
            nc.gpsimd.dma_start(out=t, in_=wo_d[kf * 128:(kf + 1) * 128, :])
            wo_t.append(t)

        # --- static tiles: V (with ones blocks), Q^T (zero-padded), K^T ---
        v_t = []
        for mk in range(NK):
            t = vpool.tile([128, VW], bf16, tag="v", name=f"v{mk}")
            t4 = t[:].rearrange("p (pr two c) -> p pr two c", two=2, c=128)
            nc.gpsimd.memset(t4[:, :, :, 0:E], 1.0)
            v_t.append(t)
        qt_t = [qpool.tile([128, S], bf16, tag="q", name=f"qt{h}")
                for h in range(2 * NF)]
        kt_t = [kpool.tile([128, S], bf16, tag="k", name=f"kt{m}")
                for m in range(NF)]
        ctxt_t = [cxpool.tile([128, S], bf16, tag="cx", name=f"ctxt{m}")
                  for m in range(NF)]
        for m in range(NF):
            nc.gpsimd.memset(qt_t[2 * m][E:128, :], 0.0)
            nc.gpsimd.memset(qt_t[2 * m + 1][0:E, :], 0.0)

        # --- projection group emitters (each: 6 matmuls + DVE drain) ---
        def emit_k_group(m, nq):
            ps = pproj.tile([128, QB], f32, tag="pj", name=f"pk{m}_{nq}")
            for kd in range(ND):
                nc.tensor.matmul(
                    ps[:, :],
                    lhsT=wk_t[kd][:, m * 128:(m + 1) * 128],
                    rhs=xt_t[kd][:, nq * QB:(nq + 1) * QB],
                    start=(kd == 0), stop=(kd == ND - 1),
                )
            nc.vector.tensor_scalar_add(
                kt_t[m][:, nq * QB:(nq + 1) * QB], ps[:, :], bk_sb[:, m:m + 1])

        def emit_q_group(m, nq):
            sl = slice(nq * QB, (nq + 1) * QB)
            ps = pproj.tile([128, QB], f32, tag="pj", name=f"pq{m}_{nq}")
            for kd in range(ND):
                nc.tensor.matmul(
                    ps[:, :],
                    lhsT=wq_t[kd][:, m * 128:(m + 1) * 128],
                    rhs=xt_t[kd][:, nq * QB:(nq + 1) * QB],
                    start=(kd == 0), stop=(kd == ND - 1),
                )
            nc.vector.tensor_scalar_add(
                qt_t[2 * m][0:E, sl], ps[0:E, :], bq_sb[0:E, m:m + 1])
            nc.vector.tensor_scalar_add(
                qt_t[2 * m + 1][E:128, sl], ps[E:128, :], bq_sb[E:128, m:m + 1])

        def emit_v_group(mk):
            ps = pproj.tile([128, F], f32, tag="pj", name=f"pv{mk}",
                            padded_shape=[128, QB])
            for kd in range(ND):
                nc.tensor.matmul(
                    ps[:, :],
                    lhsT=xt_t[kd][:, mk * 128:(mk + 1) * 128],
                    rhs=wv_t[kd][:, :],
                    start=(kd == 0), stop=(kd == ND - 1),
                )
            t4 = v_t[mk][:].rearrange("p (pr two c) -> p pr two c", two=2, c=128)
            ps3 = ps.rearrange("p (pr hh e) -> p pr hh e", hh=2, e=E)
            bv3 = bv_bc.rearrange("p (pr hh e) -> p pr hh e", hh=2, e=E)
            nc.vector.tensor_tensor(
                t4[:, :, 0, E:128], ps3[:, :, 0, :], bv3[:, :, 0, :], op=ALU.add)
            nc.vector.tensor_tensor(
                t4[:, :, 1, E:128], ps3[:, :, 1, :], bv3[:, :, 1, :], op=ALU.add)

        def emit_o_unit(mq):
            osb = opool.tile([128, D], bf16, tag="o", name=f"ot{mq}")
            for piece, (c0, c1) in enumerate(((0, 512), (512, D))):
                ps = pproj.tile([128, c1 - c0], f32, tag="pj",
                                name=f"po{piece}_{mq}", padded_shape=[128, QB])
                for kf in range(NF):
                    nc.tensor.matmul(
                        ps[:, :], lhsT=ctxt_t[kf][:, mq * 128:(mq + 1) * 128],
                        rhs=wo_t[kf][:, c0:c1],
                        start=(kf == 0), stop=(kf == NF - 1))
                nc.vector.tensor_copy(osb[:, c0:c1], ps[:, :])
            nc.sync.dma_start(out=y_d[mq * 128:(mq + 1) * 128, :], in_=osb[:])

        fillers = []

        def pump(n=1):
            for _ in range(n):
                if fillers:
                    fillers.pop(0)()

        # --- upfront: K(pair0) fully, Q(pair0, block0), V chunks 0-1 ---
        for nq in range(NQB):
            emit_k_group(0, nq)
        emit_q_group(0, 0)
        emit_v_group(0)
        emit_v_group(1)

        # --- attention unit: head pair hp, q block nq ---
        def unit(hp, nq, pn=1):
            qsl = slice(nq * QB, (nq + 1) * QB)
            cps = [psctx.tile([128, QB], f32, tag="c", name=f"c{hp}_{nq}_{h}")
                   for h in range(2)]
            sps = [None, None]
            eb = [None, None]
            prev = [None, None]  # previous g's (esb, sps-group) per head

            def emit_scores(g):
                for hh in range(2):
                    sps[hh] = pssc.tile([128, 2 * QB], f32, tag="s",
                                        name=f"s{hp}_{nq}_{g}_{hh}")
                for j in range(2):
                    mk = 2 * g + j
                    for hh in range(2):
                        nc.tensor.matmul(
                            sps[hh][:, j * QB:(j + 1) * QB],
                            lhsT=kt_t[hp][:, mk * 128:(mk + 1) * 128],
                            rhs=qt_t[2 * hp + hh][:, qsl],
                            start=True, stop=True,
                        )

            def emit_exp(g):
                for hh in range(2):
                    e = epool.tile([128, 2 * QB], bf16, tag="e",
                                   name=f"e{hp}_{nq}_{g}_{hh}")
                    nc.scalar.activation(e[:], sps[hh][:, :], AF.Exp, scale=0.125)
                    eb[hh] = e

            def emit_ctx(g, ebs):
                for hh in range(2):
                    base = hp * PW + hh * 128  # [ones64 | V_h]
                    for j in range(2):
                        mk = 2 * g + j
                        nc.tensor.matmul(
                            cps[hh][:, :],
                            lhsT=v_t[mk][:, base:base + 128],
                            rhs=ebs[hh][:, j * QB:(j + 1) * QB],
                            start=(g == 0 and j == 0),
                            stop=(g == 7 and j == 1),
                        )

            for g in range(8):
                emit_scores(g)
                pump(pn)
                if g > 0:
                    emit_ctx(g - 1, prev)
                emit_exp(g)
                prev = list(eb)
            pump(pn)
            emit_ctx(7, prev)

            # normalize: both heads produce [den 0:64 | ctx 64:128]
            for hh in range(2):
                r = rpool.tile([E, QB], f32, tag="r", name=f"r{hp}_{nq}_{hh}")
                nc.vector.reciprocal_approx_fast(out=r[:], in_=cps[hh][0:E, :])
                nc.vector.tensor_tensor(
                    ctxt_t[hp][hh * E:(hh + 1) * E, qsl], cps[hh][E:128, :],
                    r[:], op=ALU.mult)

        # --- schedule ---
        # fillers per unit, keyed in emission order (hp outer, nq inner)
        plan = {
            (0, 0): [lambda: emit_q_group(0, 1)]
                    + [lambda m=m: emit_v_group(m) for m in range(2, 16)],
            (0, 1): [lambda: emit_k_group(1, 0), lambda: emit_q_group(0, 2)],
            (0, 2): [lambda: emit_k_group(1, 1), lambda: emit_q_group(0, 3)],
            (0, 3): [lambda: emit_k_group(1, 2), lambda: emit_k_group(1, 3),
                     lambda: emit_q_group(1, 0)],
            (1, 0): [lambda: emit_k_group(2, 0), lambda: emit_q_group(1, 1)],
            (1, 1): [lambda: emit_k_group(2, 1), lambda: emit_q_group(1, 2)],
            (1, 2): [lambda: emit_k_group(2, 2), lambda: emit_q_group(1, 3)],
            (1, 3): [lambda: emit_k_group(2, 3), lambda: emit_q_group(2, 0)],
            (2, 0): [lambda: emit_q_group(2, 1)],
            (2, 1): [lambda: emit_q_group(2, 2)]
                    + [lambda m=m: emit_o_unit(m) for m in range(0, 4)],
            (2, 2): [lambda: emit_q_group(2, 3)]
                    + [lambda m=m: emit_o_unit(m) for m in range(4, 8)],
            (2, 3): [lambda m=m: emit_o_unit(m) for m in range(8, 12)],
        }
        for hp in range(NF):
            for nq in range(NQB):
                fillers.extend(plan.get((hp, nq), []))
                unit(hp, nq, pn=2 if (hp, nq) == (0, 0) else 1)
        # drain remaining fillers, then tail output projections
        while fillers:
            pump(1)
        for mq in range(12, 16):
            emit_o_unit(mq)
        if debug:
            qt_dbg = nc.declare_dram_parameter("qt_dbg", [2 * NF * 128, S], bf16,
                                               isOutput=True)
            kt_dbg = nc.declare_dram_parameter("kt_dbg", [NF * 128, S], bf16,
                                               isOutput=True)
            v_dbg = nc.declare_dram_parameter("v_dbg", [NK * 128, VW], bf16,
                                              isOutput=True)
            cx_dbg = nc.declare_dram_parameter("cx_dbg", [NF * 128, S], bf16,
                                               isOutput=True)
            for h in range(2 * NF):
                nc.sync.dma_start(out=qt_dbg[h * 128:(h + 1) * 128, :], in_=qt_t[h][:])
            for m in range(NF):
                nc.sync.dma_start(out=kt_dbg[m * 128:(m + 1) * 128, :], in_=kt_t[m][:])
                nc.sync.dma_start(out=cx_dbg[m * 128:(m + 1) * 128, :], in_=ctxt_t[m][:])
            for mk in range(NK):
                nc.sync.dma_start(out=v_dbg[mk * 128:(mk + 1) * 128, :], in_=v_t[mk][:])
    nc.compile()
    return nc


def _get_nc():
    global _NC
    if _NC is None:
        _NC = _build()
    return _NC


def kernel(x, Wq, bq, Wk, bk, Wv, bv, Wo, bo, _trace=False):
    x = np.asarray(x, dtype=np.float32)
    Wq = np.asarray(Wq, dtype=np.float32)
    bq = np.asarray(bq, dtype=np.float32)
    Wk = np.asarray(Wk, dtype=np.float32)
    bk = np.asarray(bk, dtype=np.float32)
    Wv = np.asarray(Wv, dtype=np.float32)
    bv = np.asarray(bv, dtype=np.float32)
    Wo = np.asarray(Wo, dtype=np.float32)
    bo = np.asarray(bo, dtype=np.float32)

    nc = _get_nc()
    in_maps = []
    for c in range(NCORES):
        b = c // 2
        h0 = (c % 2) * HL
        in_maps.append({
            "xt": np.ascontiguousarray(x[b].T).astype(BF),
            "wq": np.ascontiguousarray(
                Wq[h0:h0 + HL].transpose(1, 0, 2).reshape(D, F)).astype(BF),
            "wk": np.ascontiguousarray(
                Wk[h0:h0 + HL].transpose(1, 0, 2).reshape(D, F)).astype(BF),
            "wv": np.ascontiguousarray(
                Wv[h0:h0 + HL].transpose(1, 0, 2).reshape(D, F)).astype(BF),
            "wo": np.ascontiguousarray(Wo[h0 * E:(h0 + HL) * E]).astype(BF),
            "bq": np.ascontiguousarray(bq[h0:h0 + HL].reshape(F, 1)),
            "bk": np.ascontiguousarray(bk[h0:h0 + HL].reshape(F, 1)),
            "bv": np.ascontiguousarray(bv[h0:h0 + HL].reshape(1, F)),
        })
    res = run_bass_kernel_spmd(nc, in_maps, list(range(NCORES)), trace=_trace)
    out = np.empty((B, S, D), np.float32)
    for b in range(B):
        out[b] = (res.results[2 * b]["y"].astype(np.float32)
                  + res.results[2 * b + 1]["y"].astype(np.float32)
                  + bo[None, :])
    if _trace:
        kernel.last_exec_time_ns = res.exec_time_ns
        kernel.last_results = res
    return out
